# revision 26
# baseline (speedup 1.0000x reference)
"""ANOVA kernel (order 3) for Trainium2, 8 NeuronCores, pure data parallel.

Reference computation per sample b (x: (B, F, D) fp32):
    out[b] = sum_d e3(x[b, :, d])
where e3 is the 3rd elementary symmetric polynomial over the F=40 field values.

Newton's identities replace the sequential DP over F with power sums:
    p_k[b, d] = sum_f x[b, f, d]^k          (k = 1, 2, 3)
    e3 = (p1^3 - 3*p1*p2 + 2*p3) / 6
so the kernel is a pure streaming reduce — ideal for the memory-bound regime.

Per-core mapping (batch on partitions, b = p*16 + g so the one output DMA at
the end writes 64B-contiguous runs per partition):
  - DMA x in f-chunks; every chunk is >=2 fields (sub-512B descriptors pay a
    2x DMA latency multiplier) and every tile tapers its last chunks [3, 2]
    so per-tile square/mul/matmul tails run on a small remainder.
  - ScalarE: X2 = Square(X) (bf16). VectorE: X3 = X * X2 (bf16).
  - TensorE: p1/p2/p3 via PSUM-accumulating matmuls with scaled identity
    stationaries (C*I fp32r for p1, -3C^2*I / (1/3)*I bf16 for p2/p3,
    C = 6^(-1/3)), so e3 = p1ps^2*p1ps + p1ps*p2ps + p3ps and the combine
    needs no scale ops. Matmuls are pinned with ordering-only deps: tiles
    0..3 in data-arrival order (groups interleaved per chunk) so the
    in-order PE tracks the stream; the small last tile group-sequential
    (p1 fully, then p2, then p3) so its p1/p2 stops — which gate the
    combine's long pole — land right after the last chunk.
  - Per-tile e3 combine on ScalarE (t1) + VectorE, placed where the PSUM
    stops it waits on are comfortably old: tiles 0..2 drain at slot 7 of
    the next tile; tile 3's ops are pinned into measured idle slots of
    tile 4's stream; tile 4's run post-stream on clear queues.
  - One output DMA at the very end from a persistent [128, 16] buffer.
"""

import numpy as np
from contextlib import ExitStack

import concourse.bacc as bacc
import concourse.mybir as mybir
import concourse.tile as tile
from concourse import masks
from concourse.bass_utils import run_bass_kernel_spmd
from bass_rust import add_dep_helper as bass_add_dep

N_CORES = 8
B, F, D = 16384, 40, 64
B_SHARD = B // N_CORES          # 2048 batches per core
G = B_SHARD // 128              # 16 batch groups of 128 per core

FP32 = mybir.dt.float32
FP32R = mybir.dt.float32r
BF16 = mybir.dt.bfloat16
C_P1 = 6.0 ** (-1.0 / 3.0)
TAIL_SLOT = 7                   # mul-loop slot where tiles 0..2 drain


def _dedupe_ldweights(nc):
    """Remove InstLdweights that reload the weights already resident in the PE
    array (same stationary AP, no intervening self-loading fp32/fp32r matmul).
    Waits/updates of a removed load migrate to the next PE instruction."""
    PE = mybir.EngineType.PE
    removed = 0
    for block in nc.m.functions[0].blocks:
        insts = block.instructions
        cur_sig = None
        pending_sync = []
        keep = []
        for inst in insts:
            nm = type(inst).__name__
            if pending_sync and getattr(inst, "engine", None) == PE:
                si = inst.sync_info
                if si is None:
                    si = mybir.SyncInfo(on_wait=[], on_update=[])
                    inst.sync_info = si
                for psi in pending_sync:
                    si.on_wait = list(psi.on_wait) + list(si.on_wait)
                    si.on_update = list(si.on_update) + list(psi.on_update)
                pending_sync = []
            if nm == "InstMatmult":
                wap = inst.ins[1]
                if str(wap.dtype) in ("dt.float32", "dt.float32r",
                                      "float32", "float32r"):
                    cur_sig = None  # self-loading matmul clobbers the array
            elif nm == "InstLdweights":
                wap = inst.ins[0]
                sig = (str(wap.memref), wap.offset, str(wap.ap), str(wap.dtype))
                if sig == cur_sig:
                    si = inst.sync_info
                    if si is not None and (si.on_wait or si.on_update):
                        pending_sync.append(si)
                    removed += 1
                    continue
                cur_sig = sig
            keep.append(inst)
        assert not pending_sync, "dangling sync from removed trailing ldweights"
        block.instructions = keep
    return removed


def build_nc():
    nc = bacc.Bacc("TRN2", target_bir_lowering=False, debug=False,
                   num_devices=N_CORES)
    x = nc.dram_tensor("x", [B_SHARD, F, D], FP32, kind="ExternalInput")
    out = nc.dram_tensor("out", [B_SHARD, 1], FP32, kind="ExternalOutput")

    # p-major batch split: batch b = p*16 + g, so out_r's per-partition row is
    # 16 consecutive DRAM floats (64B contiguous -> cheap single output DMA).
    x_r = x.rearrange("(p g) f d -> p g f d", g=G)
    out_r = out.rearrange("(p g) o -> p (g o)", g=G)

    with tile.TileContext(nc) as tc, ExitStack() as ctx:
        const = ctx.enter_context(tc.tile_pool(name="const", bufs=1))
        obp = ctx.enter_context(tc.tile_pool(name="obp", bufs=1))
        xp = ctx.enter_context(tc.tile_pool(name="xp", bufs=2))
        x2p = ctx.enter_context(tc.tile_pool(name="x2p", bufs=2))
        x3p = ctx.enter_context(tc.tile_pool(name="x3p", bufs=2))
        tp = ctx.enter_context(tc.tile_pool(name="tp", bufs=2))
        small = ctx.enter_context(tc.tile_pool(name="small", bufs=2))
        psum = ctx.enter_context(tc.tile_pool(name="psum", bufs=2, space="PSUM"))

        ident_bf16 = const.tile([128, 128], BF16)
        masks.make_identity(nc, ident_bf16[:])
        ident_p1 = const.tile([128, 128], FP32R)
        nc.vector.tensor_scalar_mul(ident_p1[:], ident_bf16[:], C_P1)
        ident_p2 = const.tile([128, 128], BF16)
        nc.vector.tensor_scalar_mul(ident_p2[:], ident_bf16[:],
                                    -3.0 * C_P1 * C_P1)
        ident_p3 = const.tile([128, 128], BF16)
        nc.vector.tensor_scalar_mul(ident_p3[:], ident_bf16[:], 1.0 / 3.0)

        # Persistent output accumulator; one DMA drains it at the very end.
        OB = obp.tile([128, G], FP32)

        pending = {}
        mm_chain = [None]

        def chained_mm(*args, **kwargs):
            mm = nc.tensor.matmul(*args, **kwargs)
            if mm_chain[0] is not None:
                # ordering-only edge: pin the PE stream to data-arrival order
                bass_add_dep(mm.ins, mm_chain[0].ins, sync=False,
                             reason="PE data-arrival order")
            mm_chain[0] = mm
            return mm

        def pin_after(op, anchor):
            if anchor is not None:
                bass_add_dep(op.ins, anchor.ins, sync=False,
                             reason="queue placement")
            return op

        def emit_t1(i, anchor=None):
            """ACT part of tile i's combine: t1 = (C p1)^2 straight from PSUM."""
            g0, nt, st = pending[i]
            t1 = tp.tile([128, nt, D], FP32, tag="t1")
            st["t1"] = pin_after(nc.scalar.square(t1[:], st["p1"][:]), anchor)
            st["t1t"] = t1

        def drain_steps(i):
            """Tile i's combine as five individually placeable DVE steps:
            t3 = t1 + p2ps; t4 = t3*p1ps; d-reduce both parts; add into the
            output buffer. Caller interleaves them into VectorE slack."""
            g0, nt, st = pending.pop(i)
            t1 = st["t1t"]
            t3 = tp.tile([128, nt, D], FP32, tag="t3")
            t4 = tp.tile([128, nt, D], FP32, tag="t4")
            r4 = small.tile([128, nt], FP32, tag="r4")
            r3 = small.tile([128, nt], FP32, tag="r3")
            return [
                lambda: nc.vector.tensor_add(t3[:], t1[:], st["p2"][:]),
                lambda: nc.vector.tensor_mul(t4[:], t3[:], st["p1"][:]),
                lambda: nc.vector.reduce_sum(r4[:], t4[:],
                                             axis=mybir.AxisListType.X),
                lambda: nc.vector.reduce_sum(r3[:], st["p3"][:],
                                             axis=mybir.AxisListType.X),
                lambda: nc.vector.tensor_add(OB[:, g0:g0 + nt], r4[:], r3[:]),
            ]

        def emit_tail(i, anchor=None):
            """All five combine steps back to back (mid-stream big-tile use)."""
            steps = drain_steps(i)
            first = steps[0]()
            pin_after(first, anchor)
            for s in steps[1:]:
                red = s()
            return red

        # Tile sizes in 128-batch groups: the two trailing 2-group tiles keep
        # the post-stream chain small while leaving tile 4's stream window
        # wide enough to absorb tile 3's combine in engine slack.
        tile_nts = [4, 4, 4, 2, 2]
        SPREAD_AT = 5           # mul-loop slot where tile 3's combine starts
        T1_SLOT = 2             # square-loop slot for tile 3's t1
        assert sum(tile_nts) == G
        g0 = 0
        for i, nt in enumerate(tile_nts):
            last = i == len(tile_nts) - 1
            X = xp.tile([128, nt, F, D], FP32, tag="X")
            X2 = x2p.tile([128, nt, F, D], BF16, tag="X2")
            X3 = x3p.tile([128, nt, F, D], BF16, tag="X3")
            if i == 0:
                sizes = [2, 3] + [5] * 6 + [3, 2]
            else:
                sizes = [5] * 7 + [3, 2]
            assert sum(sizes) == F
            bounds = [0]
            for s in sizes:
                bounds.append(bounds[-1] + s)
            chunks = [slice(a, b) for a, b in zip(bounds[:-1], bounds[1:])]
            for fs in chunks:
                # fp32r-typed byte copy (same bits) so the fp32r matmul sees
                # a properly-typed producer; other engines read the fp32 view.
                nc.sync.dma_start(X[:, :, fs, :].bitcast(FP32R),
                                  x_r[:, g0:g0 + nt, fs, :].bitcast(FP32R))
            sqs = []
            for ci, fs in enumerate(chunks):
                if last and ci == T1_SLOT:
                    # tile 3's t1: with the group-sequential matmul tail its
                    # p1 stop lands ~1.5us after tile 3's last chunk — well
                    # before this slot's chunk — and ACT idles here anyway.
                    emit_t1(i - 1, anchor=sqs[-1])
                sq = nc.scalar.square(X2[:, :, fs, :], X[:, :, fs, :])
                if last and ci == T1_SLOT:
                    pin_after(sq, pending[i - 1][2]["t1"])
                sqs.append(sq)
            flush = None
            if last:
                _steps = drain_steps(i - 1)
                _nflush = 2
                spread = _steps[:len(_steps) - _nflush] if _nflush else _steps
                flush = _steps[len(_steps) - _nflush:] if _nflush else []
            else:
                spread = None
            muls = []
            for ci, fs in enumerate(chunks):
                ins = None
                if not last and ci == TAIL_SLOT and i >= 1:
                    # Tiles 0..2 drain fully at slot 7 of the next tile: the
                    # PSUM stops they wait on are most of a window old, and
                    # the big-tile windows have the VectorE slack.
                    emit_t1(i - 1)
                    ins = emit_tail(i - 1)
                elif spread and SPREAD_AT <= ci < SPREAD_AT + len(spread):
                    # tile 3's combine: one step per chunk slot, soaked into
                    # the per-chunk VectorE slack of tile 4's window — early
                    # slots, so the window's last muls run data-bound. The
                    # stops it waits on are ~2us old by these slots.
                    ins = pin_after(spread[ci - SPREAD_AT](), muls[-1])
                mul = nc.vector.tensor_mul(X3[:, :, fs, :], X[:, :, fs, :],
                                           X2[:, :, fs, :])
                if ins is not None:
                    # ordering-only edge: the greedy scheduler would otherwise
                    # queue every X3 chunk (ready early) ahead of the e3 ops
                    bass_add_dep(mul.ins, ins.ins, sync=False,
                                 reason="drain prev-tile e3 before late X3")
                muls.append(mul)
            if spread:
                # steps whose slots fell past the last chunk, plus the
                # deliberately held-back trailing steps, run right after the
                # stream's final mul
                for step in spread[max(0, len(chunks) - SPREAD_AT):] + flush:
                    pin_after(step(), muls[-1])

            # Power-sum accumulation groups; broadcast output AP accumulates
            # r fields per matmul. N = r*nt*D in [256, 512] (ISA max 512;
            # fp32r moving drops to 4 cycles/row below 256).
            p1ps = psum.tile([128, nt, D], FP32, tag="p1ps")
            p2ps = psum.tile([128, nt, D], FP32, tag="p2ps")
            p3ps = psum.tile([128, nt, D], FP32, tag="p3ps")
            Xr = X[:].bitcast(FP32R)
            r = 4 if last else 512 // (nt * D)
            nmm = (F + r - 1) // r

            def bcast(ps, r):
                ap = ps[:]
                return ap.__replace__(ap=[ap.ap[0], ap.ap[1], [0, r], ap.ap[2]])

            groups = [(p1ps, ident_p1, Xr), (p2ps, ident_p2, X2),
                      (p3ps, ident_p3, X3)]

            def mm(ps, ident, src, k):
                f0 = k * r
                return chained_mm(bcast(ps, r), lhsT=ident[:],
                                  rhs=src[:, :, f0:f0 + r, :],
                                  start=(k == 0), stop=(k == nmm - 1),
                                  skip_group_check=True)

            # Hybrid order. Head (fields up to the last two chunks):
            # data-arrival interleave — the k-th matmul of each group emitted
            # chunk-major so the in-order PE queue tracks the stream. Tail
            # (matmuls touching the last two chunks): group-sequential —
            # p1's tail needs only raw X chunks, so its stop (which gates the
            # combine's t1/t3/t4 long pole) lands right after the last chunk
            # instead of behind the last chunk's square->mul->p3 chain.
            gate_of_k = [next(ci for ci, fs in enumerate(chunks)
                              if fs.start <= min(k * r + r, F) - 1 < fs.stop)
                         for k in range(nmm)]
            # Per-group tail boundary: p3's head stops one section earlier
            # (its data path X -> square -> mul lags one chunk more), so the
            # group-sequential tail — whose p1 part gates the combine's long
            # pole — starts without waiting on the last head X3 chunk.
            p3_extra = 1 if last else 0
            tail_from = {0: len(chunks) - 2, 1: len(chunks) - 2,
                         2: len(chunks) - 2 - p3_extra}
            head_gates = sorted(set(g for g in gate_of_k))
            for gate in head_gates:
                for gi, (ps, ident, src) in enumerate(groups):
                    if gate >= tail_from[gi]:
                        continue
                    for k in [k for k in range(nmm) if gate_of_k[k] == gate]:
                        mm(ps, ident, src, k)
            for gi, (ps, ident, src) in enumerate(groups):
                for k in [k for k in range(nmm) if gate_of_k[k] >= tail_from[gi]]:
                    mm(ps, ident, src, k)

            pending[i] = (g0, nt, {"p1": p1ps, "p2": p2ps, "p3": p3ps})
            g0 += nt

        # Tile 4's combine post-stream on clear queues: t1 right after the
        # last square, the DVE chain behind the last mul.
        li = len(tile_nts) - 1
        emit_t1(li, anchor=sqs[-1])
        emit_tail(li, anchor=muls[-1])
        nc.sync.dma_start(out_r[:, :], OB[:])

    _dedupe_ldweights(nc)
    nc.finalize()
    return nc


_NC_CACHE = None


def _get_nc():
    global _NC_CACHE
    if _NC_CACHE is None:
        _NC_CACHE = build_nc()
    return _NC_CACHE


def run(x: np.ndarray, **spmd_kwargs):
    """Run on 8 cores; returns (out (B,1) fp32, BassKernelResults)."""
    assert x.shape == (B, F, D), x.shape
    x = np.ascontiguousarray(x, dtype=np.float32)
    nc = _get_nc()
    in_maps = [{"x": x[i * B_SHARD:(i + 1) * B_SHARD]} for i in range(N_CORES)]
    res = run_bass_kernel_spmd(nc, in_maps, core_ids=list(range(N_CORES)),
                               **spmd_kwargs)
    out = np.concatenate([res.results[i]["out"] for i in range(N_CORES)], axis=0)
    return out, res


def kernel(x: np.ndarray) -> np.ndarray:
    out, _ = run(x)
    return out


if __name__ == "__main__":
    rng = np.random.default_rng(0)
    x = rng.standard_normal((B, F, D)).astype(np.float32)
    out = kernel(x)
    print("out", out.shape, out.dtype, out[:4, 0])
